# revision 5
# baseline (speedup 1.0000x reference)
"""BagOfWords Trainium2 kernel.

Reference computation (per batch b):
    emb    = emb_table[context]                      # (T, D) gather
    logits = emb @ W.T + b                           # (T, V)
    out[t] = (sum_{s<=t} (s+1) * logits[s]) / den[t] # weighted causal cum-avg
    den[t] = (t+1)(t+2)/2

Key identity: the weighted cumsum commutes with the GEMM:
    out[t, v] = (C[t] @ W[v]) / den[t] + b[v]
    C[t, d]   = sum_{s<=t} (s+1) * emb[s, d]
so the O(T*V) cumsum collapses onto the tiny (T, D) embedding side.
On device, per 128-token chunk c:
    CT[d, t] = sum_s Xw[s, d] * UTden[s, t]  +  carry[d] * invden[t]
with Xw = (s+1)*emb (per-partition scale on ACT), UTden[s,t] = [s<=t]/den[t]
(a host constant), carry = running column-sum of Xw over previous chunks
(K=1 matmuls).  CT comes out pre-transposed (d on partitions) = exactly the
lhsT layout the big GEMM wants.  Then out = CT.T @ W.T with W.T staged in
SBUF (12 MB, resident), evicted PSUM->SBUF on DVE and streamed to HBM.

Sharding: data-parallel over B (8 batches -> 8 cores).  Each core holds the
full emb_table (for the gather) and full W; output slices are disjoint.

Raw Bass with manual semaphores (one wait per instruction): the walrus build
in this container rejects instructions carrying multiple sem waits, which
rules out the Tile scheduler's multi-wait output.

DMA semaphore discipline: a DMA's 16 per-SDMA-engine sem increments interleave
arbitrarily with other in-flight DMAs on the same semaphore, so a summed
threshold across several outstanding DMAs can fire before a lagging engine
lands its data.  Every concurrently-outstanding DMA group therefore gets its
own semaphore, waited to exactly 16 (or one group outstanding per sem).
"""

import functools
import os
from contextlib import ExitStack

import numpy as np

import concourse.bass as bass
from concourse import mybir
from concourse.bass_utils import run_bass_kernel_spmd

B, T, V, D = 8, 1024, 8000, 384
P = 128
NCHUNK = T // P            # 8 token chunks per core
KD = D // P                # 3 contraction chunks
NV = 500                   # vocab tile (one fp32 PSUM bank)
NTILES_V = V // NV         # 16
VGRP = 4                   # vocab tiles per store group
NGRP = NTILES_V // VGRP    # 4 store groups per token chunk
NSTAGE = 3                 # output staging buffers
GEMM_BANKS = 4
F32 = mybir.dt.float32
F32R = mybir.dt.float32r

# const-block column layout (single DMA, single sem)
C_POS = 0                  # [128, 8]
C_ONES = C_POS + NCHUNK    # [128, 1]
C_UTDEN = C_ONES + 1       # [128, 1024]
C_INVDEN = C_UTDEN + T     # row 0, [1, 1024]
C_ONESROW = C_INVDEN + T   # row 0, [1, 128]
C_BIAS = C_ONESROW + P     # row 0, [1, 8000] (only when has_bias)
CW_NOBIAS = C_BIAS
CW_BIAS = C_BIAS + V


def _build(has_bias: bool, gemm_f32r: bool):
    nc = bass.Bass("TRN2", target_bir_lowering=False, debug=False)

    mmdt = F32R if gemm_f32r else F32
    CW = CW_BIAS if has_bias else CW_NOBIAS

    idx_d = nc.dram_tensor("idx", [P, NCHUNK], mybir.dt.int32, kind="ExternalInput")
    table_d = nc.dram_tensor("table", [V, D], F32, kind="ExternalInput")
    wt_d = nc.dram_tensor("wt", [D, V], mmdt, kind="ExternalInput")
    consts_d = nc.dram_tensor("consts", [P, CW], F32, kind="ExternalInput")
    out_d = nc.dram_tensor("out", [T, V], F32, kind="ExternalOutput")

    with ExitStack() as ctx:
        e = ctx.enter_context
        # SBUF
        idx_sb = e(nc.sbuf_tensor("idx_sb", [P, NCHUNK], mybir.dt.int32))
        cst = e(nc.sbuf_tensor("cst", [P, CW], F32))
        emb_sb = e(nc.sbuf_tensor("emb_sb", [P, NCHUNK * D], F32))
        carry_sb = e(nc.sbuf_tensor("carry_sb", [1, NCHUNK * D], F32))
        ct_sb = [e(nc.sbuf_tensor(f"ct{k}", [P, T], mmdt)) for k in range(KD)]
        wt_sb = [e(nc.sbuf_tensor(f"wt{k}", [P, V], mmdt)) for k in range(KD)]
        ostg = [e(nc.sbuf_tensor(f"ostg{q}", [P, VGRP * NV], F32)) for q in range(NSTAGE)]
        # PSUM (8 banks total)
        gps = [e(nc.psum_tensor(f"gps{i}", [P, NV], F32)) for i in range(GEMM_BANKS)]
        ctps = [e(nc.psum_tensor(f"ctps{i}", [P, P], F32)) for i in range(2)]
        csps = [e(nc.psum_tensor(f"csps{i}", [1, D], F32)) for i in range(2)]
        # sems -- one per concurrently-outstanding DMA group
        csem = e(nc.semaphore("csem"))
        wsem = [e(nc.semaphore(f"wsem{k}")) for k in range(KD)]
        gidx = e(nc.semaphore("gidx"))
        gsem = [e(nc.semaphore(f"gsem{c}")) for c in range(NCHUNK)]
        osem = [e(nc.semaphore(f"osem{q}")) for q in range(NSTAGE)]
        # engine-progress sems (single-inc, exactly ordered)
        xsem = e(nc.semaphore("xsem"))      # pos-scales done
        pecs = e(nc.semaphore("pecs"))      # colsum matmuls
        carrys = e(nc.semaphore("carrys"))  # carry adds on DVE
        ctdone = e(nc.semaphore("ctdone"))  # prefix psum tiles done
        ctsb = e(nc.semaphore("ctsb"))      # CT psum->sbuf copies
        pegemm = e(nc.semaphore("pegemm"))  # gemm psum tiles done
        evict = e(nc.semaphore("evict"))    # gemm evictions on DVE
        blk = e(nc.Block())

        def emb_c(c):
            return emb_sb[:, c * D:(c + 1) * D]

        pos_ap = lambda c: cst[:, C_POS + c:C_POS + c + 1]
        ones_ap = cst[:, C_ONES:C_ONES + 1]
        utden_ap = lambda c: cst[:, C_UTDEN + c * P:C_UTDEN + (c + 1) * P]
        invden_ap = lambda c: cst[0:1, C_INVDEN + c * P:C_INVDEN + (c + 1) * P]
        onesrow_ap = cst[0:1, C_ONESROW:C_ONESROW + P]
        bias_ap = lambda n: cst[0:1, C_BIAS + n * NV:C_BIAS + (n + 1) * NV]

        @blk.sync
        def _(sync):
            sync.dma_start(cst[:], consts_d[:]).then_inc(csem, 16)
            for k in range(KD):
                sync.dma_start(wt_sb[k][:], wt_d[k * P:(k + 1) * P, :]).then_inc(wsem[k], 16)
            for m in range(NCHUNK):
                for g in range(NGRP):
                    gi = m * NGRP + g
                    sync.wait_ge(evict, (m * NTILES_V) + (g + 1) * VGRP)
                    sync.dma_start(
                        out_d[m * P:(m + 1) * P, g * VGRP * NV:(g + 1) * VGRP * NV],
                        ostg[gi % NSTAGE][:],
                    ).then_inc(osem[gi % NSTAGE], 16)
            for q in range(NSTAGE):
                ngrp_q = (NCHUNK * NGRP - q + NSTAGE - 1) // NSTAGE
                sync.wait_ge(osem[q], 16 * ngrp_q)

        @blk.gpsimd
        def _(gpsimd):
            gpsimd.dma_start(idx_sb[:], idx_d[:]).then_inc(gidx, 16)
            gpsimd.wait_ge(gidx, 16)
            for c in range(NCHUNK):
                gpsimd.indirect_dma_start(
                    out=emb_c(c),
                    out_offset=None,
                    in_=table_d[:],
                    in_offset=bass.IndirectOffsetOnAxis(ap=idx_sb[:, c:c + 1], axis=0),
                ).then_inc(gsem[c], 16)

        @blk.scalar
        def _(scalar):
            scalar.wait_ge(csem, 16)
            for c in range(NCHUNK):
                scalar.wait_ge(gsem[c], 16)
                scalar.mul(emb_c(c), emb_c(c), pos_ap(c)).then_inc(xsem, 1)
            for c in range(NCHUNK):
                for k in range(KD):
                    j = c * KD + k
                    scalar.wait_ge(ctdone, j + 1)
                    scalar.copy(ct_sb[k][:, c * P:(c + 1) * P], ctps[j % 2][:]).then_inc(ctsb, 1)

        @blk.tensor
        def _(tensor):
            tensor.wait_ge(csem, 16)
            # prefix stage
            for c in range(NCHUNK):
                tensor.wait_ge(xsem, c + 1)
                if c < NCHUNK - 1:
                    if c >= 2:
                        tensor.wait_ge(carrys, c - 1)  # WAR on csps[c % 2]
                    tensor.matmul(csps[c % 2][:], lhsT=ones_ap, rhs=emb_c(c),
                                  start=True, stop=True).then_inc(pecs, 1)
                if c > 0:
                    tensor.wait_ge(carrys, c)
                for k in range(KD):
                    j = c * KD + k
                    if j >= 2:
                        tensor.wait_ge(ctsb, j - 1)  # WAR on ctps[j % 2]
                    mm = tensor.matmul(
                        ctps[j % 2][:],
                        lhsT=emb_sb[:, c * D + k * P: c * D + (k + 1) * P],
                        rhs=utden_ap(c),
                        start=True, stop=(c == 0))
                    if c > 0:
                        mm = tensor.matmul(
                            ctps[j % 2][:],
                            lhsT=carry_sb[0:1, c * D + k * P: c * D + (k + 1) * P],
                            rhs=invden_ap(c),
                            start=False, stop=True)
                    mm.then_inc(ctdone, 1)
            # big GEMM
            for m in range(NCHUNK):
                tensor.wait_ge(ctsb, KD * (m + 1))
                for n in range(NTILES_V):
                    i = m * NTILES_V + n
                    if i >= GEMM_BANKS:
                        tensor.wait_ge(evict, i - GEMM_BANKS + 1)
                    for k in range(KD):
                        if i == 0:
                            tensor.wait_ge(wsem[k], 16)
                        last = (k == KD - 1) and not has_bias
                        mm = tensor.matmul(
                            gps[i % GEMM_BANKS][:],
                            lhsT=ct_sb[k][:, m * P:(m + 1) * P],
                            rhs=wt_sb[k][:, n * NV:(n + 1) * NV],
                            start=(k == 0), stop=last)
                    if has_bias:
                        mm = tensor.matmul(
                            gps[i % GEMM_BANKS][:],
                            lhsT=onesrow_ap,
                            rhs=bias_ap(n),
                            start=False, stop=True)
                    mm.then_inc(pegemm, 1)

        @blk.vector
        def _(vector):
            for c in range(NCHUNK - 1):
                vector.wait_ge(pecs, c + 1)
                dst = carry_sb[0:1, (c + 1) * D:(c + 2) * D]
                if c == 0:
                    vector.tensor_copy(dst, csps[0][:]).then_inc(carrys, 1)
                else:
                    vector.tensor_add(dst, carry_sb[0:1, c * D:(c + 1) * D],
                                      csps[c % 2][:]).then_inc(carrys, 1)
            for i in range(NCHUNK * NTILES_V):
                g = i // VGRP
                vector.wait_ge(pegemm, i + 1)
                if i % VGRP == 0 and g >= NSTAGE:
                    vector.wait_ge(osem[g % NSTAGE], 16 * (g // NSTAGE))
                vector.tensor_copy(
                    ostg[g % NSTAGE][:, (i % VGRP) * NV:(i % VGRP + 1) * NV],
                    gps[i % GEMM_BANKS][:]).then_inc(evict, 1)

    return nc


@functools.lru_cache(maxsize=None)
def _get_program(has_bias: bool, gemm_f32r: bool):
    return _build(has_bias, gemm_f32r)


@functools.lru_cache(maxsize=None)
def _host_consts(has_bias: bool):
    CW = CW_BIAS if has_bias else CW_NOBIAS
    cst = np.zeros((P, CW), dtype=np.float32)
    t = np.arange(T, dtype=np.float64)
    den = (t + 1.0) * (t + 2.0) / 2.0
    invden = (1.0 / den).astype(np.float32)
    cst[:, C_POS:C_POS + NCHUNK] = (
        np.arange(T, dtype=np.float32) + 1.0).reshape(NCHUNK, P).T
    cst[:, C_ONES] = 1.0
    s = np.arange(P)
    ut = (s[:, None] <= s[None, :]).astype(np.float32)  # [s, t_local]
    for c in range(NCHUNK):
        cst[:, C_UTDEN + c * P:C_UTDEN + (c + 1) * P] = (
            ut * invden[c * P:(c + 1) * P][None, :])
    cst[0, C_INVDEN:C_INVDEN + T] = invden
    cst[0, C_ONESROW:C_ONESROW + P] = 1.0
    return cst


GEMM_F32R = os.environ.get("BOW_F32R", "1") == "1"  # fp32r: 4x fp32 PE throughput


def kernel(context, emb_table, W, b):
    context = np.asarray(context)
    emb_table = np.ascontiguousarray(np.asarray(emb_table, dtype=np.float32))
    W = np.asarray(W, dtype=np.float32)
    b = np.asarray(b, dtype=np.float32)
    has_bias = bool(np.any(b))

    wt = np.ascontiguousarray(W.T)  # (D, V)
    cst = _host_consts(has_bias)
    if has_bias:
        cst = cst.copy()
        cst[0, C_BIAS:C_BIAS + V] = b
    nc = _get_program(has_bias, GEMM_F32R)

    in_maps = []
    for i in range(B):
        idx = np.ascontiguousarray(
            context[i].reshape(NCHUNK, P).T.astype(np.int32))  # [p, c]
        in_maps.append({"idx": idx, "table": emb_table, "wt": wt, "consts": cst})

    res = run_bass_kernel_spmd(nc, in_maps, list(range(B)))
    out = np.stack([res.results[i]["out"] for i in range(B)], axis=0)
    return out


# revision 6
# speedup vs baseline: 20.9763x; 20.9763x over previous
"""BagOfWords Trainium2 kernel.

Reference computation (per batch b):
    emb    = emb_table[context]                      # (T, D) gather
    logits = emb @ W.T + b                           # (T, V)
    out[t] = (sum_{s<=t} (s+1) * logits[s]) / den[t] # weighted causal cum-avg
    den[t] = (t+1)(t+2)/2

Key identity: the weighted cumsum commutes with the GEMM:
    out[t, v] = (C[t] @ W[v]) / den[t] + b[v]
    C[t, d]   = sum_{s<=t} (s+1) * emb[s, d]
so the O(T*V) cumsum collapses onto the tiny (T, D) embedding side.
On device, per 128-token chunk c:
    CT[d, t] = sum_s Xw[s, d] * UTden[s, t]  +  carry[d] * invden[t]
with Xw = (s+1)*emb (per-partition scale on ACT), UTden[s,t] = [s<=t]/den[t]
(a host constant), carry = running column-sum of Xw over previous chunks
(K=1 matmuls).  CT comes out pre-transposed (d on partitions) = exactly the
lhsT layout the big GEMM wants.  Then out = CT.T @ W.T with W.T staged in
SBUF (12 MB, resident), evicted PSUM->SBUF on DVE and streamed to HBM.

Sharding: data-parallel over B (8 batches -> 8 cores).  Each core holds the
full emb_table (for the gather) and full W; output slices are disjoint.

Raw Bass with manual semaphores (one wait per instruction): the walrus build
in this container rejects instructions carrying multiple sem waits, which
rules out the Tile scheduler's multi-wait output.

DMA semaphore discipline: a DMA's 16 per-SDMA-engine sem increments interleave
arbitrarily with other in-flight DMAs on the same semaphore, so a summed
threshold across several outstanding DMAs can fire before a lagging engine
lands its data.  Every concurrently-outstanding DMA group therefore gets its
own semaphore, waited to exactly 16 per iteration.

reps>1 repeats the whole pipeline inside one NEFF (used only for timing: the
benchmark fits a line over reps to cancel the ~50-100 ms axon dispatch
overhead).  Iterations re-gather from the table so every rep computes
identical values; cross-iteration WAR hazards get explicit waits.
"""

import functools
import os
from contextlib import ExitStack

import numpy as np

import concourse.bass as bass
from concourse import mybir
from concourse.bass_utils import run_bass_kernel_spmd

B, T, V, D = 8, 1024, 8000, 384
P = 128
NCHUNK = T // P            # 8 token chunks per core
KD = D // P                # 3 contraction chunks
NV = 500                   # vocab tile (one fp32 PSUM bank)
NTILES_V = V // NV         # 16
VGRP = 4                   # vocab tiles per store group
NGRP = NTILES_V // VGRP    # 4 store groups per token chunk
NSTAGE = 3                 # output staging buffers
GEMM_BANKS = 4
F32 = mybir.dt.float32
F32R = mybir.dt.float32r

# const-block column layout (single DMA, single sem)
C_POS = 0                  # [128, 8]
C_ONES = C_POS + NCHUNK    # [128, 1]
C_UTDEN = C_ONES + 1       # [128, 1024]
C_INVDEN = C_UTDEN + T     # row 0, [1, 1024]
C_ONESROW = C_INVDEN + T   # row 0, [1, 128]
C_BIAS = C_ONESROW + P     # row 0, [1, 8000] (only when has_bias)
CW_NOBIAS = C_BIAS
CW_BIAS = C_BIAS + V

# per-iteration semaphore increments
X_IT = NCHUNK              # xsem (pos scales)
CS_IT = NCHUNK - 1         # pecs (colsum matmuls)
CA_IT = NCHUNK - 1         # carrys (carry adds)
CT_IT = NCHUNK * KD        # ctdone / ctsb
GM_IT = NCHUNK * NTILES_V  # pegemm / evict
GR_IT = NCHUNK * NGRP      # store groups


def _build(has_bias: bool, gemm_f32r: bool, reps: int = 1):
    nc = bass.Bass("TRN2", target_bir_lowering=False, debug=False)

    mmdt = F32R if gemm_f32r else F32
    CW = CW_BIAS if has_bias else CW_NOBIAS

    idx_d = nc.dram_tensor("idx", [P, NCHUNK], mybir.dt.int32, kind="ExternalInput")
    table_d = nc.dram_tensor("table", [V, D], F32, kind="ExternalInput")
    wt_d = nc.dram_tensor("wt", [D, V], mmdt, kind="ExternalInput")
    consts_d = nc.dram_tensor("consts", [P, CW], F32, kind="ExternalInput")
    out_d = nc.dram_tensor("out", [T, V], F32, kind="ExternalOutput")

    with ExitStack() as ctx:
        e = ctx.enter_context
        # SBUF
        idx_sb = e(nc.sbuf_tensor("idx_sb", [P, NCHUNK], mybir.dt.int32))
        cst = e(nc.sbuf_tensor("cst", [P, CW], F32))
        emb_sb = e(nc.sbuf_tensor("emb_sb", [P, NCHUNK * D], F32))
        carry_sb = e(nc.sbuf_tensor("carry_sb", [1, NCHUNK * D], F32))
        ct_sb = [e(nc.sbuf_tensor(f"ct{k}", [P, T], mmdt)) for k in range(KD)]
        wt_sb = [e(nc.sbuf_tensor(f"wt{k}", [P, V], mmdt)) for k in range(KD)]
        ostg = [e(nc.sbuf_tensor(f"ostg{q}", [P, VGRP * NV], F32)) for q in range(NSTAGE)]
        # PSUM (8 banks total)
        gps = [e(nc.psum_tensor(f"gps{i}", [P, NV], F32)) for i in range(GEMM_BANKS)]
        ctps = [e(nc.psum_tensor(f"ctps{i}", [P, P], F32)) for i in range(2)]
        csps = [e(nc.psum_tensor(f"csps{i}", [1, D], F32)) for i in range(2)]
        # sems -- one per concurrently-outstanding DMA group
        csem = e(nc.semaphore("csem"))
        wsem = [e(nc.semaphore(f"wsem{k}")) for k in range(KD)]
        gidx = e(nc.semaphore("gidx"))
        gsem = [e(nc.semaphore(f"gsem{c}")) for c in range(NCHUNK)]
        osem = [e(nc.semaphore(f"osem{q}")) for q in range(NSTAGE)]
        # engine-progress sems (single-inc, exactly ordered)
        xsem = e(nc.semaphore("xsem"))      # pos-scales done
        pecs = e(nc.semaphore("pecs"))      # colsum matmuls
        carrys = e(nc.semaphore("carrys"))  # carry adds on DVE
        ctdone = e(nc.semaphore("ctdone"))  # prefix psum tiles done
        ctsb = e(nc.semaphore("ctsb"))      # CT psum->sbuf copies
        pegemm = e(nc.semaphore("pegemm"))  # gemm psum tiles done
        evict = e(nc.semaphore("evict"))    # gemm evictions on DVE
        blk = e(nc.Block())

        def emb_c(c):
            return emb_sb[:, c * D:(c + 1) * D]

        pos_ap = lambda c: cst[:, C_POS + c:C_POS + c + 1]
        ones_ap = cst[:, C_ONES:C_ONES + 1]
        utden_ap = lambda c: cst[:, C_UTDEN + c * P:C_UTDEN + (c + 1) * P]
        invden_ap = lambda c: cst[0:1, C_INVDEN + c * P:C_INVDEN + (c + 1) * P]
        onesrow_ap = cst[0:1, C_ONESROW:C_ONESROW + P]
        bias_ap = lambda n: cst[0:1, C_BIAS + n * NV:C_BIAS + (n + 1) * NV]

        @blk.sync
        def _(sync):
            sync.dma_start(cst[:], consts_d[:]).then_inc(csem, 16)
            for k in range(KD):
                sync.dma_start(wt_sb[k][:], wt_d[k * P:(k + 1) * P, :]).then_inc(wsem[k], 16)
            for it in range(reps):
                for m in range(NCHUNK):
                    for g in range(NGRP):
                        gi = it * GR_IT + m * NGRP + g
                        sync.wait_ge(evict, it * GM_IT + m * NTILES_V + (g + 1) * VGRP)
                        sync.dma_start(
                            out_d[m * P:(m + 1) * P, g * VGRP * NV:(g + 1) * VGRP * NV],
                            ostg[gi % NSTAGE][:],
                        ).then_inc(osem[gi % NSTAGE], 16)
            for q in range(NSTAGE):
                ngrp_q = (reps * GR_IT - q + NSTAGE - 1) // NSTAGE
                sync.wait_ge(osem[q], 16 * ngrp_q)

        @blk.gpsimd
        def _(gpsimd):
            gpsimd.dma_start(idx_sb[:], idx_d[:]).then_inc(gidx, 16)
            gpsimd.wait_ge(gidx, 16)
            for it in range(reps):
                for c in range(NCHUNK):
                    if it > 0:
                        # WAR: PE must be done reading emb chunk c of iter it-1
                        gpsimd.wait_ge(ctdone, (it - 1) * CT_IT + (c + 1) * KD)
                    gpsimd.indirect_dma_start(
                        out=emb_c(c),
                        out_offset=None,
                        in_=table_d[:],
                        in_offset=bass.IndirectOffsetOnAxis(ap=idx_sb[:, c:c + 1], axis=0),
                    ).then_inc(gsem[c], 16)

        @blk.scalar
        def _(scalar):
            scalar.wait_ge(csem, 16)
            for it in range(reps):
                for c in range(NCHUNK):
                    scalar.wait_ge(gsem[c], 16 * (it + 1))
                    scalar.mul(emb_c(c), emb_c(c), pos_ap(c)).then_inc(xsem, 1)
                for c in range(NCHUNK):
                    for k in range(KD):
                        j = it * CT_IT + c * KD + k
                        scalar.wait_ge(ctdone, j + 1)
                        if it > 0 and c == 0 and k == 0:
                            # WAR: gemm of iter it-1 must be done reading ct_sb
                            scalar.wait_ge(pegemm, it * GM_IT)
                        scalar.copy(ct_sb[k][:, c * P:(c + 1) * P],
                                    ctps[j % 2][:]).then_inc(ctsb, 1)

        @blk.tensor
        def _(tensor):
            tensor.wait_ge(csem, 16)
            for it in range(reps):
                # prefix stage
                for c in range(NCHUNK):
                    tensor.wait_ge(xsem, it * X_IT + c + 1)
                    if c < NCHUNK - 1:
                        j_cs = it * CS_IT + c
                        if j_cs >= 2:
                            tensor.wait_ge(carrys, j_cs - 1)  # WAR on csps
                        tensor.matmul(csps[c % 2][:], lhsT=ones_ap, rhs=emb_c(c),
                                      start=True, stop=True).then_inc(pecs, 1)
                    if c > 0:
                        tensor.wait_ge(carrys, it * CA_IT + c)
                    for k in range(KD):
                        j = it * CT_IT + c * KD + k
                        if j >= 2:
                            tensor.wait_ge(ctsb, j - 1)  # WAR on ctps
                        mm = tensor.matmul(
                            ctps[j % 2][:],
                            lhsT=emb_sb[:, c * D + k * P: c * D + (k + 1) * P],
                            rhs=utden_ap(c),
                            start=True, stop=(c == 0))
                        if c > 0:
                            mm = tensor.matmul(
                                ctps[j % 2][:],
                                lhsT=carry_sb[0:1, c * D + k * P: c * D + (k + 1) * P],
                                rhs=invden_ap(c),
                                start=False, stop=True)
                        mm.then_inc(ctdone, 1)
                # big GEMM
                for m in range(NCHUNK):
                    tensor.wait_ge(ctsb, it * CT_IT + KD * (m + 1))
                    for n in range(NTILES_V):
                        i = it * GM_IT + m * NTILES_V + n
                        if i >= GEMM_BANKS:
                            tensor.wait_ge(evict, i - GEMM_BANKS + 1)
                        for k in range(KD):
                            if i == 0:
                                tensor.wait_ge(wsem[k], 16)
                            last = (k == KD - 1) and not has_bias
                            mm = tensor.matmul(
                                gps[i % GEMM_BANKS][:],
                                lhsT=ct_sb[k][:, m * P:(m + 1) * P],
                                rhs=wt_sb[k][:, n * NV:(n + 1) * NV],
                                start=(k == 0), stop=last)
                        if has_bias:
                            mm = tensor.matmul(
                                gps[i % GEMM_BANKS][:],
                                lhsT=onesrow_ap,
                                rhs=bias_ap(n),
                                start=False, stop=True)
                        mm.then_inc(pegemm, 1)

        @blk.vector
        def _(vector):
            for it in range(reps):
                for c in range(NCHUNK - 1):
                    vector.wait_ge(pecs, it * CS_IT + c + 1)
                    if it > 0 and c == 0:
                        # WAR: PE carry matmuls of iter it-1 must be done
                        vector.wait_ge(ctdone, it * CT_IT)
                    dst = carry_sb[0:1, (c + 1) * D:(c + 2) * D]
                    if c == 0:
                        vector.tensor_copy(dst, csps[0][:]).then_inc(carrys, 1)
                    else:
                        vector.tensor_add(dst, carry_sb[0:1, c * D:(c + 1) * D],
                                          csps[c % 2][:]).then_inc(carrys, 1)
                for i0 in range(GM_IT):
                    i = it * GM_IT + i0
                    g = i // VGRP
                    vector.wait_ge(pegemm, i + 1)
                    if i % VGRP == 0 and g >= NSTAGE:
                        vector.wait_ge(osem[g % NSTAGE], 16 * (g // NSTAGE))
                    vector.tensor_copy(
                        ostg[g % NSTAGE][:, (i % VGRP) * NV:(i % VGRP + 1) * NV],
                        gps[i % GEMM_BANKS][:]).then_inc(evict, 1)

    return nc


@functools.lru_cache(maxsize=None)
def _get_program(has_bias: bool, gemm_f32r: bool, reps: int = 1):
    return _build(has_bias, gemm_f32r, reps)


@functools.lru_cache(maxsize=None)
def _host_consts(has_bias: bool):
    CW = CW_BIAS if has_bias else CW_NOBIAS
    cst = np.zeros((P, CW), dtype=np.float32)
    t = np.arange(T, dtype=np.float64)
    den = (t + 1.0) * (t + 2.0) / 2.0
    invden = (1.0 / den).astype(np.float32)
    cst[:, C_POS:C_POS + NCHUNK] = (
        np.arange(T, dtype=np.float32) + 1.0).reshape(NCHUNK, P).T
    cst[:, C_ONES] = 1.0
    s = np.arange(P)
    ut = (s[:, None] <= s[None, :]).astype(np.float32)  # [s, t_local]
    for c in range(NCHUNK):
        cst[:, C_UTDEN + c * P:C_UTDEN + (c + 1) * P] = (
            ut * invden[c * P:(c + 1) * P][None, :])
    cst[0, C_INVDEN:C_INVDEN + T] = invden
    cst[0, C_ONESROW:C_ONESROW + P] = 1.0
    return cst


GEMM_F32R = os.environ.get("BOW_F32R", "1") == "1"  # fp32r: 4x fp32 PE throughput


def make_in_maps(context, emb_table, W, b):
    context = np.asarray(context)
    emb_table = np.ascontiguousarray(np.asarray(emb_table, dtype=np.float32))
    W = np.asarray(W, dtype=np.float32)
    b = np.asarray(b, dtype=np.float32)
    has_bias = bool(np.any(b))

    wt = np.ascontiguousarray(W.T)  # (D, V)
    cst = _host_consts(has_bias)
    if has_bias:
        cst = cst.copy()
        cst[0, C_BIAS:C_BIAS + V] = b

    in_maps = []
    for i in range(B):
        idx = np.ascontiguousarray(
            context[i].reshape(NCHUNK, P).T.astype(np.int32))  # [p, c]
        in_maps.append({"idx": idx, "table": emb_table, "wt": wt, "consts": cst})
    return in_maps, has_bias


def kernel(context, emb_table, W, b):
    in_maps, has_bias = make_in_maps(context, emb_table, W, b)
    nc = _get_program(has_bias, GEMM_F32R)
    res = run_bass_kernel_spmd(nc, in_maps, list(range(B)))
    return np.stack([res.results[i]["out"] for i in range(B)], axis=0)
